# revision 9
# baseline (speedup 1.0000x reference)
"""GRU encoder step (embedding lookup + GRUCell, batch=1) on 8 TRN2 cores.

Sharding: core k computes hidden dims [32k, 32k+32) of h_new; the host
concatenates the 8 slices. The embedding table is replicated to every core.

Lookup path (no SWDGE): the Sync engine reg_loads the token offset
(idx*HID, host-precomputed) straight from DRAM into a register, then issues
a one-packet HWDGE DMA of table1d[0:1, ds(off, 256)] into a single SBUF
partition (x1). The Tensor engine broadcasts x1
to 96 partitions with a K=1 matmul into PSUM, which the gx contraction
reads directly. A dummy DMA issued first on Sync warms its DMA ring so the
one-packet gather's ring-fetch latency is short.

Per-core packed operand wb [128, 775] f32 (one DMA, issued by Scalar):
  partition map: p0:32 = r gate rows, p32:64 = i_n rows, p64:96 = z rows,
                 p96:128 = h_n rows (w_ih rows: r=0:256, z=256:512, n=512:768)
  cols 0:256    x-side weights (rows r/i_n/z; p96:128 zero)
  cols 258:514  h-side weights (r, zero, z, h_n)
  col 514       b_ih (r/i_n/z) and b_hh[n] on p96:128 (pairs with ones 773)
  col 515       b_hh (r and z rows; pairs with ones col 774)
  col 516       hs = this core's h slice on p64:96 (for q = z*hs)
  cols 517:773  h replicated on all 128 partitions
  cols 773:775  1.0 (ones pair for the gh bias columns; col 773 also
                serves as the "1" operand of u = 1-z)

Contractions (vector, fused mul+reduce):
  gh[128] = sum(wb[:,258:516] * wb[:,517:775])  -> Wh.h + b_ih + b_hh terms
  gx[96]  = sum(wb[0:96,0:256] * psum_x)        -> Wx.x (pure)
Vector also: i_n0 = gx[32:64] + gh[32:64] (tanh bias), off critical path.
Scalar: rz = sigmoid(gx[0:96] + gh[0:96]) (r at p0:32, z at p64:96; p32:64
  is a harmless garbage lane), then n = tanh(rz_r * gh[96:128] + i_n0)
  using the activation scale/bias APs.
Vector: u = 1-z, q = z*hs, out = n*u + q -> out_sb[32,1].
Sync: waits, DMAs out_sb to DRAM (lands during teardown).
GpSimd: memsets ones_row [1,96] at t0 (matmul lhsT + warm-act source).
"""

import os
import sys

import numpy as np

for _p in ("/opt/trn_rl_repo",):
    if _p not in sys.path and os.path.isdir(_p):
        sys.path.insert(0, _p)

import concourse.bass as bass
from concourse import mybir

VOCAB = 100000
HID = 256
NCORES = 8
G = HID // NCORES  # 32
WB_W = 775

_cached = None


def build_program():
    nc = bass.Bass(
        "TRN2",
        target_bir_lowering=False,
        debug=False,
        num_devices=NCORES,
    )
    f32 = mybir.dt.float32
    i32 = mybir.dt.int32

    table = nc.dram_tensor("table", [1, VOCAB * HID], f32, kind="ExternalInput").ap()
    wb_d = nc.dram_tensor("wb", [128, WB_W], f32, kind="ExternalInput").ap()
    idx_d = nc.dram_tensor("idx", [1, 1], i32, kind="ExternalInput").ap()
    out_d = nc.dram_tensor("out", [G, 1], f32, kind="ExternalOutput").ap()

    wb_sb = nc.alloc_sbuf_tensor("wb_sb", [128, WB_W], f32).ap()
    x1 = nc.alloc_sbuf_tensor("x1", [1, HID], f32).ap()
    ones_row = nc.alloc_sbuf_tensor("ones_row", [1, 96], f32).ap()
    dummy_sb = nc.alloc_sbuf_tensor("dummy_sb", [1, 1], i32).ap()
    psum_x = nc.alloc_psum_tensor("psum_x", [96, HID], f32).ap()
    s1 = nc.alloc_sbuf_tensor("s1", [96, HID], f32).ap()
    s2 = nc.alloc_sbuf_tensor("s2", [128, HID + 2], f32).ap()
    gx = nc.alloc_sbuf_tensor("gx", [96, 1], f32).ap()
    gh = nc.alloc_sbuf_tensor("gh", [128, 1], f32).ap()
    rz_t = nc.alloc_sbuf_tensor("rz_t", [96, 1], f32).ap()
    in0_t = nc.alloc_sbuf_tensor("in0_t", [G, 1], f32).ap()
    n_t = nc.alloc_sbuf_tensor("n_t", [G, 1], f32).ap()
    u_t = nc.alloc_sbuf_tensor("u_t", [G, 1], f32).ap()
    q_t = nc.alloc_sbuf_tensor("q_t", [G, 1], f32).ap()
    out_sb = nc.alloc_sbuf_tensor("out_sb", [G, 1], f32).ap()
    warm = nc.alloc_sbuf_tensor("warm", [1, 96], f32).ap()

    with (
        nc.semaphore() as s_w,
        nc.semaphore() as s_x,
        nc.semaphore() as s_g,
        nc.semaphore() as s_t,
        nc.semaphore() as s_v,
        nc.semaphore() as s_s,
        nc.semaphore() as s_o,
        nc.Block() as block,
    ):

        @block.sync
        def _(sync):
            # Warm this engine's DMA ring while the TENSOR_LOADs below run,
            # so the gather's ring-fetch latency is short.
            sync.dma_start(dummy_sb[:], idx_d[0:1, 0:1]).then_inc(s_o, 16)
            with sync.register("ridx") as ridx:
                sync.reg_load(ridx, idx_d[0:1, 0:1])
                off = sync.snap(ridx, min_val=0, max_val=(VOCAB - 1) * HID)
                sync.dma_start(
                    x1[0:1, 0:HID],
                    table[0:1, bass.ds(off, HID)],
                ).then_inc(s_x, 16)
            sync.wait_ge(s_v, 6)
            # No completion wait: lands during the exit barrier/teardown.
            sync.dma_start(out_d[:], out_sb[:]).then_inc(s_o, 16)

        @block.gpsimd
        def _(gpsimd):
            # matmul lhsT (broadcast vector) + warm-act source
            gpsimd.memset(ones_row[:], 1.0).then_inc(s_g, 1)

        @block.tensor
        def _(tensor):
            tensor.wait_ge(s_g, 1)
            tensor.wait_ge(s_x, 16)
            # broadcast x1 across 96 partitions: ones[1,96].T @ x1[1,256]
            tensor.matmul(
                out=psum_x[:],
                lhsT=ones_row[:],
                rhs=x1[:],
                start=True,
                stop=True,
            ).then_inc(s_t, 1)

        @block.vector
        def _(vector):
            vector.wait_ge(s_w, 16)
            # h-side contraction (+ all biases via ones cols): gh = Wh.h + b
            vector.scalar_tensor_tensor(
                out=s2[:],
                in0=wb_sb[:, 258 : 258 + HID + 2],
                scalar=1.0,
                in1=wb_sb[:, 517 : 517 + HID + 2],
                op0=mybir.AluOpType.mult,
                op1=mybir.AluOpType.mult,
                accum_out=gh[:],
            ).then_inc(s_v, 1)
            vector.wait_ge(s_t, 1)
            # x-side contraction for r/i_n/z rows (reads PSUM broadcast)
            vector.scalar_tensor_tensor(
                out=s1[:],
                in0=wb_sb[0:96, 0:HID],
                scalar=1.0,
                in1=psum_x[:],
                op0=mybir.AluOpType.mult,
                op1=mybir.AluOpType.mult,
                accum_out=gx[:],
            ).then_inc(s_v, 1)
            vector.wait_ge(s_v, 2)  # sem edge for gx (same-engine RAW)
            # tanh bias: i_n = gx_in + b_ihn (b_ihn lives in gh[32:64])
            vector.tensor_tensor(
                out=in0_t[0:G],
                in0=gx[G : 2 * G],
                in1=gh[G : 2 * G],
                op=mybir.AluOpType.add,
            ).then_inc(s_v, 1)
            vector.wait_ge(s_s, 1)  # rz ready (z at p64:96)
            vector.scalar_tensor_tensor(
                out=u_t[0:G],
                in0=rz_t[64:96],
                scalar=-1.0,
                in1=wb_sb[64:96, 773:774],
                op0=mybir.AluOpType.mult,
                op1=mybir.AluOpType.add,
            ).then_inc(s_v, 1)
            vector.tensor_tensor(
                out=q_t[0:G],
                in0=rz_t[64:96],
                in1=wb_sb[64:96, 516:517],
                op=mybir.AluOpType.mult,
            ).then_inc(s_v, 1)
            vector.wait_ge(s_s, 2)  # n ready
            vector.wait_ge(s_v, 5)  # sem edge for u_t/q_t (same-engine RAW)
            vector.scalar_tensor_tensor(
                out=out_sb[:],
                in0=n_t[:],
                scalar=u_t[0:G, :1],
                in1=q_t[0:G, :1],
                op0=mybir.AluOpType.mult,
                op1=mybir.AluOpType.add,
            ).then_inc(s_v, 1)

        @block.scalar
        def _(scalar):
            scalar.dma_start(wb_sb[:], wb_d[:]).then_inc(s_w, 16)
            # Warm the ACT table while the DMAs are in flight.
            scalar.wait_ge(s_g, 1)
            scalar.activation(
                warm[:], ones_row[:], mybir.ActivationFunctionType.Sigmoid
            )
            scalar.wait_ge(s_v, 2)  # gh and gx ready
            # r and z sigmoids in one shot (p32:64 lane is unused garbage)
            scalar.activation(
                rz_t[:],
                gx[:, :1],
                mybir.ActivationFunctionType.Sigmoid,
                bias=gh[0:96, :1],
            ).then_inc(s_s, 1)
            scalar.wait_ge(s_s, 1)  # sem edge for rz_t (same-engine RAW)
            scalar.wait_ge(s_v, 3)  # i_n0 ready
            # n = tanh(r * ghn + i_n0)
            scalar.activation(
                n_t[:],
                rz_t[0:G, :1],
                mybir.ActivationFunctionType.Tanh,
                scale=gh[96:128, :1],
                bias=in0_t[0:G, :1],
            ).then_inc(s_s, 1)

    return nc


def shard_inputs(
    input, hidden, embedding, w_ih, w_hh, b_ih, b_hh
) -> list[dict[str, np.ndarray]]:
    """Host-side marshaling: slice/replicate full inputs into per-core maps."""
    idx = int(np.asarray(input).reshape(-1)[0])
    h = np.asarray(hidden, dtype=np.float32).reshape(HID)
    table = np.ascontiguousarray(np.asarray(embedding, dtype=np.float32)).reshape(
        1, VOCAB * HID
    )
    w_ih = np.asarray(w_ih, dtype=np.float32)
    w_hh = np.asarray(w_hh, dtype=np.float32)
    b_ih = np.asarray(b_ih, dtype=np.float32)
    b_hh = np.asarray(b_hh, dtype=np.float32)

    idx_arr = np.full((1, 1), idx * HID, dtype=np.int32)

    in_maps = []
    for k in range(NCORES):
        lo = G * k
        r_sl = slice(lo, lo + G)
        z_sl = slice(HID + lo, HID + lo + G)
        n_sl = slice(2 * HID + lo, 2 * HID + lo + G)
        wb = np.zeros((128, WB_W), dtype=np.float32)
        # x-side weights
        wb[0:G, 0:HID] = w_ih[r_sl]
        wb[G : 2 * G, 0:HID] = w_ih[n_sl]
        wb[2 * G : 3 * G, 0:HID] = w_ih[z_sl]
        # h-side weights
        wb[0:G, 258 : 258 + HID] = w_hh[r_sl]
        wb[2 * G : 3 * G, 258 : 258 + HID] = w_hh[z_sl]
        wb[3 * G : 4 * G, 258 : 258 + HID] = w_hh[n_sl]
        # gh bias pair (cols 514/515 vs ones cols 773/774)
        wb[0:G, 514] = b_ih[r_sl]
        wb[G : 2 * G, 514] = b_ih[n_sl]
        wb[2 * G : 3 * G, 514] = b_ih[z_sl]
        wb[3 * G : 4 * G, 514] = b_hh[n_sl]
        wb[0:G, 515] = b_hh[r_sl]
        wb[2 * G : 3 * G, 515] = b_hh[z_sl]
        # hs on z partitions (for q = z*hs)
        wb[2 * G : 3 * G, 516] = h[r_sl]
        # h replicated + ones pair
        wb[:, 517 : 517 + HID] = h[None, :]
        wb[:, 773:775] = 1.0

        in_maps.append({"table": table, "wb": wb, "idx": idx_arr})
    return in_maps


def unshard_output(results: list[dict[str, np.ndarray]]):
    h_new = np.concatenate(
        [np.asarray(results[k]["out"]).reshape(G) for k in range(NCORES)]
    ).astype(np.float32)
    out = h_new.reshape(1, 1, HID)
    return out, out


def _get_program():
    global _cached
    if _cached is None:
        _cached = build_program()
    return _cached


def kernel(**inputs):
    from concourse.bass_utils import run_bass_kernel_spmd

    nc = _get_program()
    in_maps = shard_inputs(**inputs)
    res = run_bass_kernel_spmd(nc, in_maps, core_ids=list(range(NCORES)))
    return unshard_output(res.results)


def run_traced(**inputs):
    """Like kernel() but with NTFF tracing; returns (output, BassKernelResults)."""
    from concourse.bass_utils import run_bass_kernel_spmd

    nc = _get_program()
    in_maps = shard_inputs(**inputs)
    res = run_bass_kernel_spmd(nc, in_maps, core_ids=list(range(NCORES)), trace=True)
    return unshard_output(res.results), res
